# revision 8
# baseline (speedup 1.0000x reference)
"""Trainium2 Bass kernel for CryptoRWKV TimeMix.

Strategy: data-parallel over B (8 batches -> 8 NeuronCores). Each core runs the
full per-batch pipeline on-chip with no DRAM bounce of intermediates:

  phase A: volatility gating + time-shift mixing in transposed (C, T) layout
           (time on the free dim makes the 1-step shift a free-dim offset)
  phase B: r/k/v projections straight out of PSUM into SBUF in the layouts the
           recurrence needs (rT/kT feature-major, v natural, k natural via PE
           transpose of kT)
  phase C: chunked linear-attention recurrence (chunk=256, no intra-chunk mask)
           + per-head groupnorm, all fused in SBUF
  phase D: output projection, written back transposed (bias per-partition)

Weight foldings (host side, exact):
  - per-head decay w = exp(-exp(time_decay)) folded into Wr columns
  - gn_w folded into Wo rows; gn_b @ Wo.T becomes an output bias vector

T is processed in 4 quarters of 512 (2 chunks each) to bound SBUF usage; the
recurrence state and the time-shift boundary column carry across quarters.
"""

import sys
import os

for _p in ("/opt/trn_rl_repo", "/root/.axon_site/_ro/trn_rl_repo"):
    if os.path.isdir(_p) and _p not in sys.path:
        sys.path.insert(0, _p)

import numpy as np
import ml_dtypes

B, T, C, H = 8, 2048, 1024, 16
N = C // H            # 64
CHUNK = 256
QT = 512              # quarter of T processed per outer iteration
NQ = T // QT          # 4
JC = QT // CHUNK      # 2 chunks per quarter
KC = C // 128         # 8 contraction tiles
NG = C // 128         # 8 output-channel groups (2 heads each)

BF16 = ml_dtypes.bfloat16

_CACHE = {}


def _build_bass(w_per_head):
    import concourse.bacc as bacc
    import concourse.mybir as mybir
    from concourse.tile import TileContext
    from contextlib import ExitStack

    f32 = mybir.dt.float32
    bf16 = mybir.dt.bfloat16
    Alu = mybir.AluOpType
    Act = mybir.ActivationFunctionType

    nc = bacc.Bacc("TRN2", target_bir_lowering=False)

    xT_d = nc.dram_tensor("xT", [C, T], f32, kind="ExternalInput")
    vol_d = nc.dram_tensor("vol", [1, T], bf16, kind="ExternalInput")
    wvol_d = nc.dram_tensor("wvol", [1, C], bf16, kind="ExternalInput")
    wrts_d = nc.dram_tensor("wrts", [128, KC, C], bf16, kind="ExternalInput")
    wkt_d = nc.dram_tensor("wkt", [128, KC, C], bf16, kind="ExternalInput")
    wvt_d = nc.dram_tensor("wvt", [128, KC, C], bf16, kind="ExternalInput")
    wots_d = nc.dram_tensor("wots", [128, KC, C], bf16, kind="ExternalInput")
    maak_d = nc.dram_tensor("maak", [128, KC], f32, kind="ExternalInput")
    maav_d = nc.dram_tensor("maav", [128, KC], f32, kind="ExternalInput")
    maar_d = nc.dram_tensor("maar", [128, KC], f32, kind="ExternalInput")
    bvol_d = nc.dram_tensor("bvolt", [128, KC], f32, kind="ExternalInput")
    obias_d = nc.dram_tensor("obiast", [128, NG], f32, kind="ExternalInput")
    ident_d = nc.dram_tensor("ident", [128, 128], bf16, kind="ExternalInput")
    outT_d = nc.dram_tensor("outT", [C, T], f32, kind="ExternalOutput")

    with TileContext(nc) as tc, ExitStack() as ctx:
        cp = ctx.enter_context(tc.tile_pool(name="consts", bufs=1))
        bp = ctx.enter_context(tc.tile_pool(name="big", bufs=1))
        wp = ctx.enter_context(tc.tile_pool(name="work", bufs=3))
        sp = ctx.enter_context(tc.tile_pool(name="small", bufs=4))
        pbig = ctx.enter_context(tc.tile_pool(name="pbig", bufs=2, space="PSUM"))
        patt = ctx.enter_context(tc.tile_pool(name="patt", bufs=2, space="PSUM"))
        psm = ctx.enter_context(tc.tile_pool(name="psm", bufs=4, space="PSUM"))

        # ---- constants ----
        wrts = cp.tile([128, KC, C], bf16, name="wrts")
        nc.sync.dma_start(wrts[:], wrts_d[:])
        wkt = cp.tile([128, KC, C], bf16, name="wkt")
        nc.sync.dma_start(wkt[:], wkt_d[:])
        wvt = cp.tile([128, KC, C], bf16, name="wvt")
        nc.sync.dma_start(wvt[:], wvt_d[:])
        wots = cp.tile([128, KC, C], bf16, name="wots")
        nc.sync.dma_start(wots[:], wots_d[:])
        maak = cp.tile([128, KC], f32, name="maak")
        nc.sync.dma_start(maak[:], maak_d[:])
        maav = cp.tile([128, KC], f32, name="maav")
        nc.sync.dma_start(maav[:], maav_d[:])
        maar = cp.tile([128, KC], f32, name="maar")
        nc.sync.dma_start(maar[:], maar_d[:])
        bvolt = cp.tile([128, KC], f32, name="bvolt")
        nc.sync.dma_start(bvolt[:], bvol_d[:])
        obiast = cp.tile([128, NG], f32, name="obiast")
        nc.sync.dma_start(obiast[:], obias_d[:])
        ident = cp.tile([128, 128], bf16, name="ident")
        nc.sync.dma_start(ident[:], ident_d[:])
        volr = cp.tile([1, T], bf16, name="volr")
        nc.sync.dma_start(volr[:], vol_d[:])
        wvolr = cp.tile([1, C], bf16, name="wvolr")
        nc.sync.dma_start(wvolr[:], wvol_d[:])

        epsb = cp.tile([128, 1], f32, name="epsb")
        nc.vector.memset(epsb[:], 1e-5)
        state = cp.tile([128, NG, N], f32, name="state")
        nc.vector.memset(state[:], 0.0)
        stateb = cp.tile([128, NG, N], bf16, name="stateb")
        nc.vector.memset(stateb[:], 0.0)
        prevcol = cp.tile([128, KC], f32, name="prevcol")
        nc.vector.memset(prevcol[:], 0.0)

        copy_i = 0

        def copy(dst, src):
            nonlocal copy_i
            copy_i += 1
            if copy_i % 2:
                nc.scalar.copy(dst, src)
            else:
                nc.vector.tensor_copy(dst, src)

        for q in range(NQ):
            t0 = q * QT

            # ---------------- phase A: gating + time-shift mixes ----------
            xk = bp.tile([128, KC, QT], bf16, name="xk", tag="xk")
            xv = bp.tile([128, KC, QT], bf16, name="xv", tag="xv")
            xr = bp.tile([128, KC, QT], bf16, name="xr", tag="xr")
            for kc in range(KC):
                xin = wp.tile([128, QT], f32, name="xin", tag="xin")
                nc.sync.dma_start(xin[:], xT_d[kc * 128:(kc + 1) * 128, t0:t0 + QT])
                pz = pbig.tile([128, QT], f32, name="pz", tag="pp")
                nc.tensor.matmul(
                    pz[:],
                    lhsT=wvolr[0:1, kc * 128:(kc + 1) * 128],
                    rhs=volr[0:1, t0:t0 + QT],
                    start=True, stop=True,
                )
                vemb = wp.tile([128, QT], f32, name="vemb", tag="vemb")
                nc.scalar.activation(vemb[:], pz[:], Act.Sigmoid,
                                     bias=bvolt[:, kc:kc + 1], scale=1.0)
                xg = wp.tile([128, QT + 1], f32, name="xg", tag="xg")
                nc.vector.tensor_copy(xg[:, 0:1], prevcol[:, kc:kc + 1])
                nc.vector.tensor_mul(xg[:, 1:QT + 1], xin[:], vemb[:])
                nc.scalar.copy(prevcol[:, kc:kc + 1], xg[:, QT:QT + 1])
                xx = wp.tile([128, QT], f32, name="xx", tag="xx")
                nc.vector.tensor_sub(xx[:], xg[:, 0:QT], xg[:, 1:QT + 1])
                nc.vector.scalar_tensor_tensor(
                    out=xk[:, kc, :], in0=xx[:], scalar=maak[:, kc:kc + 1],
                    in1=xg[:, 1:QT + 1], op0=Alu.mult, op1=Alu.add)
                nc.vector.scalar_tensor_tensor(
                    out=xv[:, kc, :], in0=xx[:], scalar=maav[:, kc:kc + 1],
                    in1=xg[:, 1:QT + 1], op0=Alu.mult, op1=Alu.add)
                nc.vector.scalar_tensor_tensor(
                    out=xr[:, kc, :], in0=xx[:], scalar=maar[:, kc:kc + 1],
                    in1=xg[:, 1:QT + 1], op0=Alu.mult, op1=Alu.add)

            # ---------------- phase B: projections -----------------------
            rT = bp.tile([128, NG, QT], bf16, name="rT", tag="rT")
            kT = bp.tile([128, NG, QT], bf16, name="kT", tag="kT")
            for g in range(NG):
                pr = pbig.tile([128, QT], f32, name="pr", tag="pp")
                for kc in range(KC):
                    nc.tensor.matmul(
                        pr[:], lhsT=wrts[:, kc, g * 128:(g + 1) * 128],
                        rhs=xr[:, kc, :], start=(kc == 0), stop=(kc == KC - 1))
                copy(rT[:, g, :], pr[:])
                pk = pbig.tile([128, QT], f32, name="pk", tag="pp")
                for kc in range(KC):
                    nc.tensor.matmul(
                        pk[:], lhsT=wkt[:, kc, g * 128:(g + 1) * 128],
                        rhs=xk[:, kc, :], start=(kc == 0), stop=(kc == KC - 1))
                copy(kT[:, g, :], pk[:])

            vnat = bp.tile([128, QT // 128, C], bf16, name="vnat", tag="vnat")
            for tt in range(QT // 128):
                for half in range(2):
                    pv = pbig.tile([128, 512], f32, name="pv", tag="pp")
                    for kc in range(KC):
                        nc.tensor.matmul(
                            pv[:], lhsT=xv[:, kc, tt * 128:(tt + 1) * 128],
                            rhs=wvt[:, kc, half * 512:(half + 1) * 512],
                            start=(kc == 0), stop=(kc == KC - 1))
                    copy(vnat[:, tt, half * 512:(half + 1) * 512], pv[:])

            knat = bp.tile([128, QT // 128, C], bf16, name="knat", tag="knat")
            for tt in range(QT // 128):
                for g in range(NG):
                    pkn = psm.tile([128, 128], bf16, name="pkn", tag="sm")
                    nc.tensor.transpose(
                        pkn[:], kT[:, g, tt * 128:(tt + 1) * 128], ident[:])
                    copy(knat[:, tt, g * 128:(g + 1) * 128], pkn[:])

            # ---------------- phase C: WKV recurrence + groupnorm ---------
            yhatT = bp.tile([128, NG, QT], bf16, name="yhatT", tag="yhatT")
            for j in range(JC):
                c0 = j * CHUNK      # chunk offset within the quarter
                # all heads' att first: no dependence on state, keeps PE busy
                att_sbs = []
                for h in range(H):
                    g, hh = h // 2, h % 2
                    po = hh * 64
                    pa = patt.tile([128, 2, CHUNK], f32, name="pa", tag="att")
                    for sh in range(2):
                        nc.tensor.matmul(
                            pa[:, sh, :],
                            lhsT=kT[po:po + 64, g, c0 + sh * 128:c0 + (sh + 1) * 128],
                            rhs=rT[po:po + 64, g, c0:c0 + CHUNK],
                            start=True, stop=True)
                    attsb = sp.tile([128, 2, CHUNK], bf16, name="attsb", tag="attsb",
                                    bufs=6)
                    copy(attsb[:], pa[:])
                    att_sbs.append(attsb)

                for g in range(NG):
                    yh2 = [sp.tile([128, 128], bf16, name="yh2", tag=f"yh2_{tt2}")
                           for tt2 in range(2)]
                    for hh in range(2):
                        h = 2 * g + hh
                        po = hh * 64
                        wh = float(w_per_head[h])
                        attsb = att_sbs[h]
                        py = psm.tile([128, 2, N], f32, name="py", tag="sm")
                        for tt2 in range(2):
                            nc.tensor.matmul(
                                py[:, tt2, :],
                                lhsT=rT[po:po + 64, g, c0 + tt2 * 128:c0 + (tt2 + 1) * 128],
                                rhs=stateb[po:po + 64, g, :],
                                start=True, stop=False)
                            for sh in range(2):
                                nc.tensor.matmul(
                                    py[:, tt2, :],
                                    lhsT=attsb[:, sh, tt2 * 128:(tt2 + 1) * 128],
                                    rhs=vnat[:, 2 * j + sh, h * 64:(h + 1) * 64],
                                    start=False, stop=(sh == 1))
                        # state update: state = w*state + w*(k^T v)
                        psd = psm.tile([128, N], f32, name="psd", tag="sm")
                        for sh in range(2):
                            nc.tensor.matmul(
                                psd[po:po + 64, :],
                                lhsT=knat[:, 2 * j + sh, g * 128 + po:g * 128 + po + 64],
                                rhs=vnat[:, 2 * j + sh, h * 64:(h + 1) * 64],
                                start=(sh == 0), stop=(sh == 1))
                        nc.scalar.mul(state[po:po + 64, g, :],
                                      state[po:po + 64, g, :], wh)
                        nc.vector.scalar_tensor_tensor(
                            out=state[po:po + 64, g, :], in0=psd[po:po + 64, :],
                            scalar=wh, in1=state[po:po + 64, g, :],
                            op0=Alu.mult, op1=Alu.add)
                        nc.scalar.copy(stateb[po:po + 64, g, :],
                                       state[po:po + 64, g, :])
                        # groupnorm over the head dim (free axis)
                        for tt2 in range(2):
                            st6 = sp.tile([128, 6], f32, name="st6", tag="st6")
                            nc.vector.bn_stats(st6[:], py[:, tt2, :])
                            st2 = sp.tile([128, 2], f32, name="st2", tag="st2")
                            nc.vector.bn_aggr(st2[:], st6[:])
                            stdv = sp.tile([128, 1], f32, name="stdv", tag="stdv")
                            nc.scalar.activation(stdv[:], st2[:, 1:2], Act.Sqrt,
                                                 bias=epsb[:])
                            rstd = sp.tile([128, 1], f32, name="rstd", tag="rstd")
                            nc.vector.reciprocal(rstd[:], stdv[:])
                            nc.vector.tensor_scalar(
                                yh2[tt2][:, po:po + 64], py[:, tt2, :],
                                st2[:, 0:1], rstd[:],
                                op0=Alu.subtract, op1=Alu.mult)
                    for tt2 in range(2):
                        pyt = psm.tile([128, 128], bf16, name="pyt", tag="sm")
                        nc.tensor.transpose(pyt[:], yh2[tt2][:], ident[:])
                        copy(yhatT[:, g, c0 + tt2 * 128:c0 + (tt2 + 1) * 128],
                             pyt[:])

            # ---------------- phase D: output projection ------------------
            for g in range(NG):
                po_ = pbig.tile([128, QT], f32, name="po_", tag="pp")
                for kc in range(KC):
                    nc.tensor.matmul(
                        po_[:], lhsT=wots[:, kc, g * 128:(g + 1) * 128],
                        rhs=yhatT[:, kc, :], start=(kc == 0), stop=(kc == KC - 1))
                osb = wp.tile([128, QT], f32, name="osb", tag="osb")
                nc.scalar.activation(osb[:], po_[:], Act.Identity,
                                     bias=obiast[:, g:g + 1], scale=1.0)
                nc.sync.dma_start(outT_d[g * 128:(g + 1) * 128, t0:t0 + QT], osb[:])

    nc.compile()
    return nc


def _get_nc(w_per_head):
    key = "nc"
    if key not in _CACHE:
        _CACHE[key] = _build_bass(w_per_head)
    return _CACHE[key]


def kernel(x, volatility, Wvol, bvol, time_maa_k, time_maa_v, time_maa_r,
           time_decay, Wk, Wv, Wr, Wo, gn_w, gn_b):
    from concourse.bass_utils import run_bass_kernel_spmd

    x = np.asarray(x, np.float32)
    volatility = np.asarray(volatility, np.float32)
    Wvol = np.asarray(Wvol, np.float32)
    bvol = np.asarray(bvol, np.float32)
    time_maa_k = np.asarray(time_maa_k, np.float32)
    time_maa_v = np.asarray(time_maa_v, np.float32)
    time_maa_r = np.asarray(time_maa_r, np.float32)
    time_decay = np.asarray(time_decay, np.float32)
    Wk = np.asarray(Wk, np.float32)
    Wv = np.asarray(Wv, np.float32)
    Wr = np.asarray(Wr, np.float32)
    Wo = np.asarray(Wo, np.float32)
    gn_w = np.asarray(gn_w, np.float32)
    gn_b = np.asarray(gn_b, np.float32)

    # host-side weight foldings (tiny, exact)
    w = np.exp(-np.exp(time_decay.astype(np.float64))).astype(np.float32)  # (H,)
    wvec = np.repeat(w, N)                        # (C,)
    WrTs = Wr.T * wvec[None, :]                   # (C_in, C_out)
    WkT = np.ascontiguousarray(Wk.T)
    WvT = np.ascontiguousarray(Wv.T)
    WoTs = Wo.T * gn_w[:, None]
    obias = (gn_b @ Wo.T).astype(np.float32)      # (C,)

    def wtile(a):  # (C_in, C_out) -> [128, KC, C] bf16
        return np.ascontiguousarray(
            a.reshape(KC, 128, C).transpose(1, 0, 2)).astype(BF16)

    def ptile(v):  # (C,) -> [128, KC] f32
        return np.ascontiguousarray(v.reshape(KC, 128).T).astype(np.float32)

    common = {
        "wvol": Wvol[:, 0][None, :].astype(BF16),
        "wrts": wtile(WrTs),
        "wkt": wtile(WkT),
        "wvt": wtile(WvT),
        "wots": wtile(WoTs),
        "maak": ptile(time_maa_k),
        "maav": ptile(time_maa_v),
        "maar": ptile(time_maa_r),
        "bvolt": ptile(bvol),
        "obiast": ptile(obias),
        "ident": np.eye(128, dtype=np.float32).astype(BF16),
    }

    nc = _get_nc(w)
    in_maps = []
    for b in range(B):
        m = dict(common)
        m["xT"] = np.ascontiguousarray(x[b].T)
        m["vol"] = volatility[b][None, :].astype(BF16)
        in_maps.append(m)

    res = run_bass_kernel_spmd(nc, in_maps, list(range(B)))
    out = np.stack([np.ascontiguousarray(res.results[b]["outT"].T)
                    for b in range(B)])
    return out.astype(np.float32)


if __name__ == "__main__":
    rng = np.random.default_rng(0)
    demo = {
        "x": rng.standard_normal((B, T, C), dtype=np.float32),
        "volatility": rng.random((B, T), dtype=np.float32),
        "Wvol": rng.standard_normal((C, 1), dtype=np.float32) * 0.02,
        "bvol": np.zeros((C,), np.float32),
        "time_maa_k": rng.random((C,), dtype=np.float32),
        "time_maa_v": rng.random((C,), dtype=np.float32),
        "time_maa_r": rng.random((C,), dtype=np.float32),
        "time_decay": rng.standard_normal((H,)).astype(np.float32),
        "Wk": rng.standard_normal((C, C), dtype=np.float32) * 0.02,
        "Wv": rng.standard_normal((C, C), dtype=np.float32) * 0.02,
        "Wr": rng.standard_normal((C, C), dtype=np.float32) * 0.02,
        "Wo": rng.standard_normal((C, C), dtype=np.float32) * 0.02,
        "gn_w": np.ones((C,), np.float32),
        "gn_b": np.zeros((C,), np.float32),
    }
    out = kernel(**demo)
    print("out", out.shape, out.dtype, float(np.abs(out).mean()))
